# revision 25
# baseline (speedup 1.0000x reference)
"""MAMConv1d Trainium2 kernel.

Y[b,o,l] = max_{c,k}(W[o,c,k] * x[b,c,l+k]) + min_{c,k}(...) + bias[o]
B=8, C=64, L=1024, O=64, K=3, stride=1, Lout=1022.

Strategy (8 NeuronCores, data-parallel over batch B):
- Per core b: products are formed on the TensorEngine via block-diagonal
  matmuls: out[l, k*512 + o*64 + c] = x[c, s+k+l] * W[o,c,k], using
  lhsT = x-window [64c, 128l] (stationary) and rhs = diag-expanded weights
  [64c', 512] (8 output channels per matmul, N=512).
  The k-shift is absorbed into the lhsT column offset, so a single
  free-axis reduce per o yields the max/min over all (k, c) at once.
- VectorEngine tensor_reduce (axis=XY over a [p, o, k, c] view of PSUM)
  produces Y'[l, o] max and min tiles; two adds fold them with bias.
- Output is written l-major [1024, 64] per core; host transposes/gathers.
"""

import numpy as np

_B, _C, _L = 8, 64, 1024
_O, _K = 64, 3
_LOUT = (_L - _K) + 1  # 1022
_LPAD = _L + 8  # zero-padded x columns so every matmul window is full
_OG = 8  # o-channels per matmul / reduce group
_NT = 8  # l-tiles of 128

_cache = {}

# TensorEngine input dtype for the product matmuls. float32 is 4 cyc/row on
# trn2; float32r is 1 cyc/row at N>=512 (numerics verified against the fp32
# reference in test.py).
_MM_DTYPE = "float32r"

# "fp32": two direct tensor_reduce ops over PSUM (exact, 1x rate).
# "bf16tree": ScalarE casts products to bf16 in SBUF, then the k-combine
# runs as contiguous tensor_tensor max/min at the DVE 2x bf16 rate and only
# the final c-reduce runs at 1x. ~1.4x faster overall; adds ~2e-3 rounding
# which is far inside the accuracy budget.
_REDUCE = "bf16tree"


def _build_module():
    import concourse.bacc as bacc
    import concourse.bass as bass
    import concourse.mybir as mybir
    import concourse.tile as tile

    f32 = mybir.dt.float32
    mmdt = getattr(mybir.dt, _MM_DTYPE)
    nc = bacc.Bacc("TRN2", target_bir_lowering=False, debug=False)

    x_d = nc.dram_tensor("x", [_C, _LPAD], mmdt, kind="ExternalInput")
    wd_d = nc.dram_tensor("wd", [_O // _OG, _C, _K * _OG * _C], mmdt, kind="ExternalInput")
    bias_d = nc.dram_tensor("bias_t", [128, _O], f32, kind="ExternalInput")
    yt_d = nc.dram_tensor("yt", [_NT * 128, _O], f32, kind="ExternalOutput")

    n_og = _O // _OG  # 8 groups of 8 output channels
    gcols = _K * _OG * _C  # 1536 product columns per group

    with tile.TileContext(nc) as tc:
        with (
            tc.tile_pool(name="const", bufs=1) as cpool,
            tc.tile_pool(name="psum", bufs=2, space=bass.MemorySpace.PSUM) as ppool,
            tc.tile_pool(name="outp", bufs=3) as opool,
        ):
            gsz0 = _OG * _C  # 512
            # split input DMAs finely so the first matmuls are gated on
            # ~128KB, not on whole-tensor transfers
            xs = cpool.tile([_C, _LPAD], mmdt)
            wds = [cpool.tile([_C, gcols], mmdt, name=f"wds{og}") for og in range(n_og)]
            for k in range(_K):
                cs = slice(k * gsz0, (k + 1) * gsz0)
                nc.sync.dma_start(wds[0][:, cs], wd_d[0][:, cs])
            for xi in range(4):
                cs = slice(xi * 258, min(_LPAD, (xi + 1) * 258))
                nc.sync.dma_start(xs[:, cs], x_d[:, cs])
            for og in range(1, n_og):
                for k in range(_K):
                    cs = slice(k * gsz0, (k + 1) * gsz0)
                    nc.sync.dma_start(wds[og][:, cs], wd_d[og][:, cs])
            bias_sb = cpool.tile([128, _O], f32)
            nc.sync.dma_start(bias_sb[:], bias_d[:])

            bf16 = mybir.dt.float16
            gsz = _OG * _C  # 512 columns per k-plane

            mx, mn = mybir.AluOpType.max, mybir.AluOpType.min
            X = mybir.AxisListType.X

            for t in range(_NT):
                s = 128 * t
                ymax = opool.tile([128, _O], f32, tag="ymax")
                ymin = opool.tile([128, _O], f32, tag="ymin")
                # og groups share one fp16 staging tile so the DVE ops run
                # at large FD (amortizes per-op overhead). The first l-tile
                # uses small groups so the DVE pipeline fills early.
                _J = (2, 4, 8, 8, 8, 8, 8, 8)[t]
                for og2 in range(n_og // _J):
                    # S layout: [p, k, j, o*c] (k-major) so the k-combine
                    # tensor_tensor ops read fully contiguous [128, J*512]
                    Sf = opool.tile([128, _K, 8, gsz], bf16, tag="S", bufs=2)
                    S = Sf[:, :, :_J, :]
                    for j in range(_J):
                        og = _J * og2 + j
                        P = ppool.tile([128, gcols], f32, tag="P")
                        for k in range(_K):
                            nc.tensor.matmul(
                                P[:, k * gsz : (k + 1) * gsz],
                                xs[:, s + k : s + k + 128],
                                wds[og][:, k * gsz : (k + 1) * gsz],
                            )
                        # cast to fp16, scattering the k planes
                        nc.scalar.copy(
                            S[:, :, j, :],
                            P.rearrange("p (k q) -> p k q", k=_K),
                        )
                    k0, k1, k2 = (S[:, i, :, :] for i in range(_K))
                    ng = _J * _OG  # 32 (o-channels per quad)
                    tx = opool.tile([128, _J * gsz], bf16, tag="tx", bufs=2)
                    tn = opool.tile([128, _J * gsz], bf16, tag="tn", bufs=2)
                    nc.vector.tensor_tensor(tx[:], k0, k1, op=mx)
                    nc.vector.tensor_tensor(tx[:], tx[:], k2, op=mx)
                    nc.vector.tensor_tensor(tn[:], k0, k1, op=mn)
                    nc.vector.tensor_tensor(tn[:], tn[:], k2, op=mn)
                    # c-tree: halve 64 -> 32 -> 16 at the 2x rate, then reduce
                    txv = tx.rearrange("p (g c) -> p g c", c=_C)
                    tnv = tn.rearrange("p (g c) -> p g c", c=_C)
                    ux = opool.tile([128, ng, 32], bf16, tag="ux")
                    un = opool.tile([128, ng, 32], bf16, tag="un")
                    nc.vector.tensor_tensor(ux[:], txv[:, :, 0:32], txv[:, :, 32:64], op=mx)
                    nc.vector.tensor_tensor(un[:], tnv[:, :, 0:32], tnv[:, :, 32:64], op=mn)
                    vx = opool.tile([128, ng, 16], bf16, tag="vx")
                    vn = opool.tile([128, ng, 16], bf16, tag="vn")
                    nc.vector.tensor_tensor(vx[:], ux[:, :, 0:16], ux[:, :, 16:32], op=mx)
                    nc.vector.tensor_tensor(vn[:], un[:, :, 0:16], un[:, :, 16:32], op=mn)
                    wx = opool.tile([128, ng, 8], bf16, tag="wx")
                    wn = opool.tile([128, ng, 8], bf16, tag="wn")
                    nc.vector.tensor_tensor(wx[:], vx[:, :, 0:8], vx[:, :, 8:16], op=mx)
                    nc.vector.tensor_tensor(wn[:], vn[:, :, 0:8], vn[:, :, 8:16], op=mn)
                    zx = opool.tile([128, ng, 4], bf16, tag="zx")
                    zn = opool.tile([128, ng, 4], bf16, tag="zn")
                    nc.vector.tensor_tensor(zx[:], wx[:, :, 0:4], wx[:, :, 4:8], op=mx)
                    nc.vector.tensor_tensor(zn[:], wn[:, :, 0:4], wn[:, :, 4:8], op=mn)
                    oslc = slice(og2 * ng, (og2 + 1) * ng)
                    nc.vector.tensor_reduce(ymax[:, oslc], zx[:], axis=X, op=mx)
                    nc.vector.tensor_reduce(ymin[:, oslc], zn[:], axis=X, op=mn)
                ysum = opool.tile([128, _O], f32, tag="ysum")
                # gpsimd adds overlap with DVE mid-stream; the last tile's
                # adds go on DVE so the kernel tail stays short
                eng = nc.vector if t >= _NT - 2 else nc.gpsimd
                eng.tensor_add(ysum[:], ymax[:], ymin[:])
                eng.tensor_add(ysum[:], ysum[:], bias_sb[:])
                nc.sync.dma_start(yt_d[s : s + 128, :], ysum[:])

    nc.compile()
    return nc


def _get_module():
    if "nc" not in _cache:
        _cache["nc"] = _build_module()
    return _cache["nc"]


def _pack_weights(weight):
    # wd[og, c', k*512 + oi*64 + c] = (c'==c) * weight[og*8+oi, c, k]
    wq = weight.reshape(_O // _OG, _OG, _C, _K)  # [og, oi, c, k]
    wd = np.zeros((_O // _OG, _C, _K, _OG, _C), dtype=np.float32)
    ci = np.arange(_C)
    # LHS advanced-index shape: [C, og, K, og_i]; RHS must match [c, og, k, oi]
    wd[:, ci, :, :, ci] = wq.transpose(2, 0, 3, 1)
    return np.ascontiguousarray(wd.reshape(_O // _OG, _C, _K * _OG * _C))


def kernel(x, weight, bias, stride):
    from concourse import bass_utils

    x = np.asarray(x, dtype=np.float32)
    weight = np.asarray(weight, dtype=np.float32)
    bias = np.asarray(bias, dtype=np.float32)
    assert int(stride) == 1
    assert x.shape == (_B, _C, _L) and weight.shape == (_O, _C, _K)

    nc = _get_module()

    wd = _pack_weights(weight)
    bias_t = np.ascontiguousarray(
        np.broadcast_to(bias.astype(np.float32), (128, _O))
    )
    xp = np.zeros((_B, _C, _LPAD), dtype=np.float32)
    xp[:, :, :_L] = x

    in_maps = [
        {"x": xp[b], "wd": wd, "bias_t": bias_t} for b in range(_B)
    ]
    res = bass_utils.run_bass_kernel_spmd(nc, in_maps, core_ids=list(range(_B)))
    _cache["last_results"] = res

    y = np.empty((_B, _O, _LOUT), dtype=np.float32)
    for b in range(_B):
        y[b] = res.results[b]["yt"][:_LOUT, :].T
    return y


# revision 27
# speedup vs baseline: 1.0187x; 1.0187x over previous
"""MAMConv1d Trainium2 kernel.

Y[b,o,l] = max_{c,k}(W[o,c,k] * x[b,c,l+k]) + min_{c,k}(...) + bias[o]
B=8, C=64, L=1024, O=64, K=3, stride=1, Lout=1022.

Strategy (8 NeuronCores, data-parallel over batch B):
- Per core b: products are formed on the TensorEngine via block-diagonal
  matmuls: out[l, k*512 + o*64 + c] = x[c, s+k+l] * W[o,c,k], using
  lhsT = x-window [64c, 128l] (stationary) and rhs = diag-expanded weights
  [64c', 512] (8 output channels per matmul, N=512).
  The k-shift is absorbed into the lhsT column offset, so a single
  free-axis reduce per o yields the max/min over all (k, c) at once.
- ScalarE casts each PSUM product block to fp16 in SBUF; the VectorEngine
  then combines the K planes with tensor_tensor max/min at the 2x fp16
  rate, runs a halving tree over C, and a final small tensor_reduce
  produces Y'[l, o] max/min tiles. Adds fold in bias (GpSimd mid-stream,
  DVE for the last tiles to keep the kernel tail short).
- Output is written l-major [1024, 64] per core; host transposes/gathers.
"""

import numpy as np

_B, _C, _L = 8, 64, 1024
_O, _K = 64, 3
_LOUT = (_L - _K) + 1  # 1022
_LPAD = _L + 8  # zero-padded x columns so every matmul window is full
_OG = 8  # o-channels per matmul / reduce group
_NT = 8  # l-tiles of 128

_cache = {}

# TensorEngine input dtype for the product matmuls. float32 is 4 cyc/row on
# trn2; float32r is 1 cyc/row at N>=512 (numerics verified against the fp32
# reference in test.py).
_MM_DTYPE = "float32r"

# "fp32": two direct tensor_reduce ops over PSUM (exact, 1x rate).
# "bf16tree": ScalarE casts products to fp16 in SBUF, then the k-combine
# runs as contiguous tensor_tensor max/min at the DVE 2x 16-bit rate and
# only a small final reduce runs at 1x. ~1.5x faster overall; adds ~5e-4
# rounding which is far inside the accuracy budget.
_REDUCE = "bf16tree"


def _build_module():
    import concourse.bacc as bacc
    import concourse.bass as bass
    import concourse.mybir as mybir
    import concourse.tile as tile

    f32 = mybir.dt.float32
    mmdt = getattr(mybir.dt, _MM_DTYPE)
    nc = bacc.Bacc("TRN2", target_bir_lowering=False, debug=False)

    x_d = nc.dram_tensor("x", [_C, _LPAD], mmdt, kind="ExternalInput")
    wd_d = nc.dram_tensor("wd", [_O // _OG, _C, _K * _OG * _C], mmdt, kind="ExternalInput")
    bias_d = nc.dram_tensor("bias_t", [128, _O], f32, kind="ExternalInput")
    yt_d = nc.dram_tensor("yt", [_NT * 128, _O], f32, kind="ExternalOutput")

    n_og = _O // _OG  # 8 groups of 8 output channels
    gcols = _K * _OG * _C  # 1536 product columns per group

    with tile.TileContext(nc) as tc:
        with (
            tc.tile_pool(name="const", bufs=1) as cpool,
            tc.tile_pool(name="psum", bufs=2, space=bass.MemorySpace.PSUM) as ppool,
            tc.tile_pool(name="outp", bufs=3) as opool,
        ):
            gsz0 = _OG * _C  # 512
            # split input DMAs finely so the first matmuls are gated on
            # ~128KB, not on whole-tensor transfers
            xs = cpool.tile([_C, _LPAD], mmdt)
            wds = [cpool.tile([_C, gcols], mmdt, name=f"wds{og}") for og in range(n_og)]
            for k in range(_K):
                cs = slice(k * gsz0, (k + 1) * gsz0)
                nc.sync.dma_start(wds[0][:, cs], wd_d[0][:, cs])
            for xi in range(4):
                cs = slice(xi * 258, min(_LPAD, (xi + 1) * 258))
                nc.sync.dma_start(xs[:, cs], x_d[:, cs])
            for og in range(1, n_og):
                for k in range(_K):
                    cs = slice(k * gsz0, (k + 1) * gsz0)
                    nc.sync.dma_start(wds[og][:, cs], wd_d[og][:, cs])
            bias_sb = cpool.tile([128, _O], f32)
            nc.sync.dma_start(bias_sb[:], bias_d[:])

            bf16 = mybir.dt.float16
            gsz = _OG * _C  # 512 columns per k-plane

            mx, mn = mybir.AluOpType.max, mybir.AluOpType.min
            X = mybir.AxisListType.X

            for t in range(_NT):
                s = 128 * t
                ymax = opool.tile([128, _O], f32, tag="ymax")
                ymin = opool.tile([128, _O], f32, tag="ymin")
                # og groups share one fp16 staging tile so the DVE ops run
                # at large FD (amortizes per-op overhead). The first l-tile
                # uses small groups so the DVE pipeline fills early.
                _J = (2, 4, 4, 8, 8, 8, 8, 8)[t]
                for og2 in range(n_og // _J):
                    # S layout: [p, k, j, o*c] (k-major) so the k-combine
                    # tensor_tensor ops read fully contiguous [128, J*512]
                    Sf = opool.tile([128, _K, 8, gsz], bf16, tag="S", bufs=2)
                    S = Sf[:, :, :_J, :]
                    for j in range(_J):
                        og = _J * og2 + j
                        P = ppool.tile([128, gcols], f32, tag="P")
                        for k in range(_K):
                            nc.tensor.matmul(
                                P[:, k * gsz : (k + 1) * gsz],
                                xs[:, s + k : s + k + 128],
                                wds[og][:, k * gsz : (k + 1) * gsz],
                            )
                        # cast to fp16, scattering the k planes
                        nc.scalar.copy(
                            S[:, :, j, :],
                            P.rearrange("p (k q) -> p k q", k=_K),
                        )
                    k0, k1, k2 = (S[:, i, :, :] for i in range(_K))
                    ng = _J * _OG  # o-channels in this group
                    tx = opool.tile([128, _J * gsz], bf16, tag="tx", bufs=2)
                    tn = opool.tile([128, _J * gsz], bf16, tag="tn", bufs=2)
                    nc.vector.tensor_tensor(tx[:], k0, k1, op=mx)
                    nc.vector.tensor_tensor(tx[:], tx[:], k2, op=mx)
                    nc.vector.tensor_tensor(tn[:], k0, k1, op=mn)
                    nc.vector.tensor_tensor(tn[:], tn[:], k2, op=mn)
                    # c-tree: halve 64 -> 32 -> 16 at the 2x rate, then reduce
                    txv = tx.rearrange("p (g c) -> p g c", c=_C)
                    tnv = tn.rearrange("p (g c) -> p g c", c=_C)
                    ux = opool.tile([128, ng, 32], bf16, tag="ux")
                    un = opool.tile([128, ng, 32], bf16, tag="un")
                    nc.vector.tensor_tensor(ux[:], txv[:, :, 0:32], txv[:, :, 32:64], op=mx)
                    nc.vector.tensor_tensor(un[:], tnv[:, :, 0:32], tnv[:, :, 32:64], op=mn)
                    vx = opool.tile([128, ng, 16], bf16, tag="vx")
                    vn = opool.tile([128, ng, 16], bf16, tag="vn")
                    nc.vector.tensor_tensor(vx[:], ux[:, :, 0:16], ux[:, :, 16:32], op=mx)
                    nc.vector.tensor_tensor(vn[:], un[:, :, 0:16], un[:, :, 16:32], op=mn)
                    wx = opool.tile([128, ng, 8], bf16, tag="wx")
                    wn = opool.tile([128, ng, 8], bf16, tag="wn")
                    nc.vector.tensor_tensor(wx[:], vx[:, :, 0:8], vx[:, :, 8:16], op=mx)
                    nc.vector.tensor_tensor(wn[:], vn[:, :, 0:8], vn[:, :, 8:16], op=mn)
                    zx = opool.tile([128, ng, 4], bf16, tag="zx")
                    zn = opool.tile([128, ng, 4], bf16, tag="zn")
                    nc.vector.tensor_tensor(zx[:], wx[:, :, 0:4], wx[:, :, 4:8], op=mx)
                    nc.vector.tensor_tensor(zn[:], wn[:, :, 0:4], wn[:, :, 4:8], op=mn)
                    oslc = slice(og2 * ng, (og2 + 1) * ng)
                    nc.vector.tensor_reduce(ymax[:, oslc], zx[:], axis=X, op=mx)
                    nc.vector.tensor_reduce(ymin[:, oslc], zn[:], axis=X, op=mn)
                ysum = opool.tile([128, _O], f32, tag="ysum")
                # gpsimd adds overlap with DVE mid-stream; the last tile's
                # adds go on DVE so the kernel tail stays short
                eng = nc.vector if t >= _NT - 2 else nc.gpsimd
                eng.tensor_add(ysum[:], ymax[:], ymin[:])
                eng.tensor_add(ysum[:], ysum[:], bias_sb[:])
                nc.sync.dma_start(yt_d[s : s + 128, :], ysum[:])

    nc.compile()
    return nc


def _get_module():
    if "nc" not in _cache:
        _cache["nc"] = _build_module()
    return _cache["nc"]


def _pack_weights(weight):
    # wd[og, c', k*512 + oi*64 + c] = (c'==c) * weight[og*8+oi, c, k]
    wq = weight.reshape(_O // _OG, _OG, _C, _K)  # [og, oi, c, k]
    wd = np.zeros((_O // _OG, _C, _K, _OG, _C), dtype=np.float32)
    ci = np.arange(_C)
    # LHS advanced-index shape: [C, og, K, og_i]; RHS must match [c, og, k, oi]
    wd[:, ci, :, :, ci] = wq.transpose(2, 0, 3, 1)
    return np.ascontiguousarray(wd.reshape(_O // _OG, _C, _K * _OG * _C))


def kernel(x, weight, bias, stride):
    from concourse import bass_utils

    x = np.asarray(x, dtype=np.float32)
    weight = np.asarray(weight, dtype=np.float32)
    bias = np.asarray(bias, dtype=np.float32)
    assert int(stride) == 1
    assert x.shape == (_B, _C, _L) and weight.shape == (_O, _C, _K)

    nc = _get_module()

    wd = _pack_weights(weight)
    bias_t = np.ascontiguousarray(
        np.broadcast_to(bias.astype(np.float32), (128, _O))
    )
    xp = np.zeros((_B, _C, _LPAD), dtype=np.float32)
    xp[:, :, :_L] = x

    in_maps = [
        {"x": xp[b], "wd": wd, "bias_t": bias_t} for b in range(_B)
    ]
    res = bass_utils.run_bass_kernel_spmd(nc, in_maps, core_ids=list(range(_B)))
    _cache["last_results"] = res

    y = np.empty((_B, _O, _LOUT), dtype=np.float32)
    for b in range(_B):
        y[b] = res.results[b]["yt"][:_LOUT, :].T
    return y
